# revision 38
# baseline (speedup 1.0000x reference)
"""Trainium2 Bass kernel for nn_Attention_4037269258732 (GQA attention with
RoPE, causal mask, and per-head sink-logit LSE renormalization).

Problem:  B=1, S=2048, DIM=2048, H=32 q-heads, KVH=8 kv-heads, HD=64.
          out = Wo @ attn(RoPE(Wq x), RoPE(Wk x), Wv x) + bo, causal,
          with out rows scaled by sigmoid(lse - sink_h).

Sharding (8 cores, tensor-parallel over heads):
  core c owns q-heads [4c, 4c+4), kv-head c, the matching rows of
  wq/wk/wv, wo's input-dim slice [256c, 256c+256), and sinks[4c:4c+4].
  Each core computes a full-shape [S, DIM] partial of the output
  projection (wo_b/8 added on every core); the host sums the 8 partials
  (that sum is the o-dim contraction of the output projection).

Device dataflow per core (feature dims on SBUF partitions so every
matmul chains without transposes; bf16 matmul operands / fp32 PSUM):
  qT[256,S], kT[64,S], vT[64,S] = W.T @ xT      (xT host-transposed)
  RoPE fused into PSUM eviction: q = (q+b)*cos + shifted(q+b)*sin_rot
  (rot_half as 32-partition-shifted DVE reads; sign folded into sin_rot)
  v_nat[S,64] via PE transpose;  Vext = [v_nat | 1]
  per (block b of 512 sq, sk-tile t, head h):
    P^T[sk,sq] = exp(kT_t.T @ q / 8)   (tri-mask on diagonal subtiles,
                                        upper-right tiles skipped)
    outT_ext[65,sq] += Vext_t.T @ P^T  (row 64 = sum_exp = softmax denom)
  per block: renorm rows r=sum_exp+e^sink at partitions {0,32,64,96},
    1/r via exp(-ln r) on ACT, broadcast via K=1 matmul,
    out_norm = outT * bcast;  then final[sq_tile, :] = outstk.T @ woT
    + wo_b/8 -> DRAM partial
"""

import numpy as np
import ml_dtypes

import bass_rust
import concourse.bass as bass
import concourse.tile as tile
from concourse import mybir
from concourse.bass_utils import run_bass_kernel_spmd

F32 = mybir.dt.float32
BF16 = mybir.dt.bfloat16
AF = mybir.ActivationFunctionType
OP = mybir.AluOpType
BF = ml_dtypes.bfloat16

B, S, DIM = 1, 2048, 2048
H, KVH, HD = 32, 8, 64
NCORES = 8
QH = H // NCORES          # 4 q heads per core
SBLK = 512                # sq block size
NSB = S // SBLK           # 4
NDC = DIM // 128          # 16 contraction chunks
NST = S // 128            # 16 sk tiles
SCALE = 1.0 / 8.0         # 1/sqrt(HD)

_ws_ctr = [0]


def _fix_range_clears(nc):
    """walrus here rejects the EVENT_SEMAPHORE_RANGE_CLEAR ISA struct
    ("ISA wrong length"); replace with per-sem write-0 NoOps."""
    import re as _re
    for f in nc.m.functions:
        for blk in f.blocks:
            out, changed = [], False
            for inst in blk.instructions:
                if type(inst).__name__ == "InstISA" and inst.isa_opcode == 176:
                    m = _re.search(r"range_first=(\d+) range_last=(\d+)", inst.concise())
                    first, last = int(m.group(1)), int(m.group(2))
                    for semid in range(first, last + 1):
                        _ws_ctr[0] += 1
                        nop = mybir.InstNoOp(name=f"I-rc-{_ws_ctr[0]}", ins=[], outs=[])
                        nop.engine = inst.engine
                        nop.sync_info = bass_rust.SyncInfo(
                            on_wait=[],
                            on_update=[
                                bass_rust.SyncUpdate(
                                    sync_type="semaphore",
                                    id=semid,
                                    update_mode="sem-wr-imm",
                                    update_value=0,
                                )
                            ],
                        )
                        out.append(nop)
                    changed = True
                    continue
                out.append(inst)
            if changed:
                blk.instructions = out


def _split_excess_waits(nc, max_waits=1):
    """walrus on this image encodes at most one SyncWait per instruction;
    hoist excess waits onto same-engine NoOps placed just before."""
    for f in nc.m.functions:
        for blk in f.blocks:
            out, changed = [], False
            for inst in blk.instructions:
                si = inst.sync_info
                waits = list(si.on_wait) if si is not None else []
                if len(waits) > max_waits:
                    excess, keep = waits[:-max_waits], waits[-max_waits:]
                    for k in range(0, len(excess), max_waits):
                        _ws_ctr[0] += 1
                        nop = mybir.InstNoOp(name=f"I-ws-{_ws_ctr[0]}", ins=[], outs=[])
                        nop.engine = inst.engine
                        nop.sync_info = bass_rust.SyncInfo(
                            on_wait=excess[k : k + max_waits], on_update=[]
                        )
                        out.append(nop)
                    inst.sync_info = bass_rust.SyncInfo(
                        on_wait=keep, on_update=list(si.on_update)
                    )
                    changed = True
                out.append(inst)
            if changed:
                blk.instructions = out


def prep_inputs(inputs):
    """Host-side sharding/layout prep. Returns per-core input maps."""
    x = np.asarray(inputs["x"], np.float32)
    rope = np.asarray(inputs["rope_cache"], np.float32)
    wq = np.asarray(inputs["wq_w"], np.float32)
    bq = np.asarray(inputs["wq_b"], np.float32)
    wk = np.asarray(inputs["wk_w"], np.float32)
    bk = np.asarray(inputs["wk_b"], np.float32)
    wv = np.asarray(inputs["wv_w"], np.float32)
    bv = np.asarray(inputs["wv_b"], np.float32)
    wo = np.asarray(inputs["wo_w"], np.float32)
    bo = np.asarray(inputs["wo_b"], np.float32)
    sinks = np.asarray(inputs["sinks"], np.float32)

    xT = np.ascontiguousarray(x[0].T).astype(BF)            # [DIM, S]
    cosT = rope[:, :HD].T                                   # [64, S]
    sinT = rope[:, HD:].T
    cos2 = np.ascontiguousarray(np.concatenate([cosT, cosT], 0)).astype(np.float32)
    # sin_rot indexed by SOURCE partition: source rows hd in [0,32) land at
    # out rows hd+32 with +sin[hd+32]; source rows hd in [32,64) land at
    # out rows hd-32 with -sin[hd-32]. Duplicated for both heads per tile.
    sr = np.concatenate([sinT[32:64], -sinT[0:32]], 0)      # [64, S]
    sin_rot2 = np.ascontiguousarray(np.concatenate([sr, sr], 0)).astype(np.float32)
    tri = np.triu(np.ones((128, 128), BF))                  # mask[p, j] = j >= p
    ident = np.eye(HD, dtype=BF)
    wob8 = (bo / NCORES).reshape(1, DIM).astype(np.float32)

    in_maps = []
    for c in range(NCORES):
        qs = slice(c * QH * HD, (c + 1) * QH * HD)          # 256 q rows
        ks = slice(c * HD, (c + 1) * HD)                    # 64 kv rows
        # wproj columns: [q 256 | k 64 | v 64] = 384
        wproj = np.concatenate([wq[qs].T, wk[ks].T, wv[ks].T], axis=1)
        bcol = np.zeros((128, 3), np.float32)
        bcol[:, 0] = bq[qs][0:128]
        bcol[:, 1] = bq[qs][128:256]
        bcol[0:64, 2] = bk[ks]
        bcol[64:128, 2] = bv[ks]
        woT = np.ascontiguousarray(wo[:, qs].T).astype(BF)  # [256, DIM]
        esink = np.tile(np.exp(sinks[c * QH : (c + 1) * QH]).reshape(1, QH),
                        (128, 1))
        in_maps.append(
            {
                "xT": xT,
                "wproj": np.ascontiguousarray(wproj).astype(BF),
                "bproj": bcol,
                "cos2": cos2,
                "sinr2": sin_rot2,
                "woT": woT,
                "wob8": wob8,
                "esink": esink.astype(np.float32),
                "tri": tri,
                "ident": ident,
                "ones_f": np.ones((128, 128), np.float32),
                "onesb": np.ones((128, 1), BF),
            }
        )
    return in_maps


def build_nc(split_waits=True):
    nc = bass.Bass("TRN2", target_bir_lowering=False, debug=False, num_devices=NCORES)
    xT = nc.dram_tensor("xT", [DIM, S], BF16, kind="ExternalInput").ap()
    wproj = nc.dram_tensor("wproj", [DIM, 384], BF16, kind="ExternalInput").ap()
    bproj = nc.dram_tensor("bproj", [128, 3], F32, kind="ExternalInput").ap()
    cos2 = nc.dram_tensor("cos2", [128, S], F32, kind="ExternalInput").ap()
    sinr2 = nc.dram_tensor("sinr2", [128, S], F32, kind="ExternalInput").ap()
    woT = nc.dram_tensor("woT", [2 * 128, DIM], BF16, kind="ExternalInput").ap()
    wob8 = nc.dram_tensor("wob8", [1, DIM], F32, kind="ExternalInput").ap()
    esink = nc.dram_tensor("esink", [128, QH], F32, kind="ExternalInput").ap()
    tri = nc.dram_tensor("tri", [128, 128], BF16, kind="ExternalInput").ap()
    ident = nc.dram_tensor("ident", [HD, HD], BF16, kind="ExternalInput").ap()
    ones_f = nc.dram_tensor("ones_f", [128, 128], F32, kind="ExternalInput").ap()
    onesb = nc.dram_tensor("onesb", [128, 1], BF16, kind="ExternalInput").ap()
    out = nc.dram_tensor("out", [S, DIM], F32, kind="ExternalOutput").ap()

    with tile.TileContext(nc) as tc:
        with tc.tile_pool(name="persist", bufs=1) as P:
            # ---- long-lived tiles ----
            esink_t = P.tile([128, QH], F32, tag="esink")
            tri_t = P.tile([128, 128], BF16, tag="tri")
            wo_t = [
                P.tile([128, DIM], BF16, name=f"wo{i}", tag=f"wo{i}")
                for i in range(2)
            ]
            biasb = P.tile([128, DIM], F32, tag="biasb")
            ones_ft = P.tile([128, 128], F32, tag="ones_ft")
            wob_row = P.tile([1, DIM], F32, tag="wobrow")
            # tiny dummy Exp/Ln to pull the ACT table load off the
            # attention critical path
            scr = P.tile([1, 16], F32, tag="scr")
            qp = [P.tile([128, S], BF16, name=f"qp{i}", tag=f"qp{i}") for i in range(2)]
            kT2 = P.tile([128, S], BF16, tag="kT2")
            vext = P.tile([128, NST * (HD + 1)], BF16, tag="vext")
            onesb_t = P.tile([128, 1], BF16, tag="onesb_t")
            outstk = [P.tile([128, S], BF16, name=f"os{i}", tag=f"os{i}") for i in range(2)]
            vT = P.tile([64, S], BF16, tag="vT")
            idp_t = P.tile([HD, HD], BF16, tag="idp")

            # ---- qkv projection, rope fused into eviction ----
            with (
                tc.tile_pool(name="projw", bufs=1) as PW,
                tc.tile_pool(name="tmp", bufs=2) as TMP,
                tc.tile_pool(name="psproj", bufs=2, space="PSUM") as PSP,
                tc.tile_pool(name="psv", bufs=2, space="PSUM") as PSV,
            ):
                x_t, w_t = [], []
                for dc in range(NDC):
                    wt = PW.tile([128, 384], BF16, name=f"w{dc}", tag=f"w{dc}")
                    nc.gpsimd.dma_start(wt[:], wproj[dc * 128 : (dc + 1) * 128, :])
                    w_t.append(wt)
                    xt = PW.tile([128, S], BF16, name=f"x{dc}", tag=f"x{dc}")
                    if dc < 2:
                        for q4 in range(4):
                            nc.sync.dma_start(
                                xt[:, q4 * SBLK : (q4 + 1) * SBLK],
                                xT[dc * 128 : (dc + 1) * 128,
                                   q4 * SBLK : (q4 + 1) * SBLK],
                            )
                    else:
                        nc.sync.dma_start(xt[:], xT[dc * 128 : (dc + 1) * 128, :])
                    x_t.append(xt)
                bcol_t = PW.tile([128, 3], F32, tag="bcol")
                nc.gpsimd.dma_start(bcol_t[:], bproj[:])
                cos_t = PW.tile([128, S], F32, tag="cos")
                nc.gpsimd.dma_start(cos_t[:], cos2[:])
                sinr_t = PW.tile([128, S], F32, tag="sinr")
                nc.gpsimd.dma_start(sinr_t[:], sinr2[:])
                id_t = PW.tile([HD, HD], BF16, tag="ident")
                nc.gpsimd.dma_start(id_t[:], ident[:])
                nc.gpsimd.dma_start(onesb_t[:], onesb[:])
                nc.gpsimd.dma_start(esink_t[:], esink[:])
                nc.gpsimd.dma_start(tri_t[:], tri[:])
                nc.gpsimd.dma_start(ones_ft[:], ones_f[:])
                nc.gpsimd.dma_start(wob_row[:], wob8[:])
                for i in range(2):
                    nc.gpsimd.dma_start(
                        wo_t[i][:], woT[i * 128 : (i + 1) * 128, :]
                    )
                nc.gpsimd.dma_start(idp_t[:], ident[:])
                nc.scalar.activation(scr[0:1, 0:3], bcol_t[0:1, 0:3], AF.Exp)
                nc.scalar.activation(scr[0:1, 0:3], scr[0:1, 0:3], AF.Ln)

                for sb in range(NSB):
                    ss = slice(sb * SBLK, (sb + 1) * SBLK)
                    ps = [
                        PSP.tile([128, SBLK], F32, name=f"pp{j}", tag=f"pp{j}")
                        for j in range(3)
                    ]
                    for dc in range(NDC):
                        for j, (c0, c1) in enumerate(
                            [(0, 128), (128, 256), (256, 384)]
                        ):
                            nc.tensor.matmul(
                                ps[j][:],
                                w_t[dc][:, c0:c1],
                                x_t[dc][:, ss],
                                start=(dc == 0),
                                stop=(dc == NDC - 1),
                            )
                    # rope eviction: cos part for both heads of a ptile at
                    # once; rot part via 32-partition-shifted reads with the
                    # sign folded into sinr_t; combine per head into qh (bf16)
                    for i in range(2):
                        t1 = TMP.tile([128, SBLK], F32, name="t1", tag="t1")
                        nc.vector.scalar_tensor_tensor(
                            t1[:], ps[i][:], bcol_t[:, i : i + 1], cos_t[:, ss],
                            op0=OP.add, op1=OP.mult,
                        )
                        t2 = TMP.tile([128, SBLK], F32, name="t2", tag="t2")
                        for g in range(4):
                            d0 = 32 * g
                            s0 = 32 * g + 32 if g % 2 == 0 else 32 * g - 32
                            nc.vector.scalar_tensor_tensor(
                                t2[d0 : d0 + 32, :],
                                ps[i][s0 : s0 + 32, :],
                                bcol_t[s0 : s0 + 32, i : i + 1],
                                sinr_t[s0 : s0 + 32, ss],
                                op0=OP.add, op1=OP.mult,
                            )
                        nc.vector.tensor_tensor(
                            qp[i][:, ss], t1[:], t2[:], op=OP.add
                        )
                    # k: rows 0:64 of ps[2]
                    tk1 = TMP.tile([64, SBLK], F32, name="tk1", tag="tk1")
                    nc.vector.scalar_tensor_tensor(
                        tk1[:], ps[2][0:64, :], bcol_t[0:64, 2:3], cos_t[0:64, ss],
                        op0=OP.add, op1=OP.mult,
                    )
                    tk2 = TMP.tile([64, SBLK], F32, name="tk2", tag="tk2")
                    nc.vector.scalar_tensor_tensor(
                        tk2[0:32, :], ps[2][32:64, :], bcol_t[32:64, 2:3],
                        sinr_t[32:64, ss], op0=OP.add, op1=OP.mult,
                    )
                    nc.vector.scalar_tensor_tensor(
                        tk2[32:64, :], ps[2][0:32, :], bcol_t[0:32, 2:3],
                        sinr_t[0:32, ss], op0=OP.add, op1=OP.mult,
                    )
                    nc.vector.tensor_tensor(
                        kT2[0:64, ss], tk1[:], tk2[:], op=OP.add
                    )
                    nc.vector.tensor_copy(kT2[64:128, ss], kT2[0:64, ss])
                    # v: rows 64:128 of ps[2], bias only
                    nc.vector.tensor_scalar_add(
                        vT[:, ss], ps[2][64:128, :], bcol_t[64:128, 2:3]
                    )
                    # transpose this block's v tiles into Vext right away
                    # (sb=3's transposes are deferred past attention block 0
                    # so the PE doesn't stall on the last rope eviction)
                    if sb < 3:
                        for t in range(4 * sb, 4 * sb + 4):
                            pv = PSV.tile([128, HD], BF16, name="pv", tag="pv")
                            nc.tensor.transpose(
                                pv[:], vT[:, t * 128 : (t + 1) * 128], id_t[:]
                            )
                            nc.vector.tensor_copy(
                                vext[:, t * 65 : t * 65 + 64], pv[:]
                            )
                            nc.vector.tensor_copy(
                                vext[:, t * 65 + 64 : t * 65 + 65], onesb_t[:]
                            )

            # ---- attention + per-block renorm + output projection ----
            with (
                tc.tile_pool(name="aux", bufs=2, space="PSUM") as AUX,
                tc.tile_pool(name="pso", bufs=1, space="PSUM") as PSO,
                tc.tile_pool(name="pss", bufs=2, space="PSUM") as PSS,
                tc.tile_pool(name="ptp", bufs=10) as PTP,
                tc.tile_pool(name="rows", bufs=2) as RP,
                tc.tile_pool(name="rbp", bufs=2) as RBP,
                tc.tile_pool(name="oev", bufs=4) as OE,
            ):
                # wo bias broadcast rows (K=1 matmuls)
                for db in range(NSB):
                    ds = slice(db * SBLK, (db + 1) * SBLK)
                    ps_bb = AUX.tile([128, SBLK], F32, name="ps_bb", tag="aux")
                    nc.tensor.matmul(
                        ps_bb[:], ones_ft[0:1, :], wob_row[0:1, ds],
                        start=True, stop=True,
                    )
                    nc.vector.tensor_copy(biasb[:, ds], ps_bb[:])
                for b in range(NSB):
                    pso = [
                        PSO.tile([65, SBLK], F32, name=f"oo{i}", tag=f"oo{i}")
                        for i in range(QH)
                    ]
                    nt = 4 * b + 4
                    for t in range(nt):
                        off = 128 * (t - 4 * b) if t >= 4 * b else 0
                        ptts = []
                        for hp in range(2):
                            # two K=64 score matmuls packed into disjoint
                            # PE row groups -> run concurrently
                            psa = PSS.tile([128, SBLK], F32, name="psa", tag="ss")
                            psb = PSS.tile([128, SBLK], F32, name="psb", tag="ss")
                            nc.tensor.matmul(
                                psa[:, off:SBLK],
                                kT2[0:64, t * 128 : (t + 1) * 128],
                                qp[hp][0:64, b * SBLK + off : (b + 1) * SBLK],
                                start=True,
                                stop=True,
                                tile_position=(0, 0),
                            )
                            nc.tensor.matmul(
                                psb[:, off:SBLK],
                                kT2[64:128, t * 128 : (t + 1) * 128],
                                qp[hp][64:128, b * SBLK + off : (b + 1) * SBLK],
                                start=True,
                                stop=True,
                                tile_position=(64, 0),
                            )
                            for lane, pss in ((0, psa), (1, psb)):
                                ptt = PTP.tile([128, SBLK], BF16, name="ptt", tag="pt")
                                nc.scalar.activation(
                                    ptt[:, off:SBLK], pss[:, off:SBLK], AF.Exp,
                                    scale=SCALE,
                                )
                                if t >= 4 * b:
                                    nc.vector.tensor_tensor(
                                        ptt[:, off : off + 128],
                                        ptt[:, off : off + 128],
                                        tri_t[:],
                                        op=OP.mult,
                                    )
                                ptts.append(ptt)
                        for h in range(QH):
                            nc.tensor.matmul(
                                pso[h][:, off:SBLK],
                                vext[:, t * 65 : (t + 1) * 65],
                                ptts[h][:, off:SBLK],
                                start=(t == 0),
                                stop=(t == nt - 1),
                            )
                    # sink renorm via exp(-ln r) + K=1 bcast matmul
                    rowb = RP.tile([128, SBLK], F32, name="rowb", tag="rowb")
                    nc.gpsimd.memset(rowb[:], 1.0)
                    for h in range(QH):
                        nc.vector.tensor_scalar_add(
                            rowb[32 * h : 32 * h + 1, :],
                            pso[h][64:65, :],
                            esink_t[64:65, h : h + 1],
                        )
                    rinvb = RP.tile([128, SBLK], F32, name="rinvb", tag="rinvb")
                    nc.scalar.activation(rinvb[:], rowb[:], AF.Ln)
                    nc.scalar.activation(rowb[:], rinvb[:], AF.Exp, scale=-1.0)
                    for h in range(QH):
                        qb = (h % 2) * 64
                        ps_rb = AUX.tile([64, SBLK], F32, name="ps_rb", tag="aux")
                        nc.tensor.matmul(
                            ps_rb[:], ones_ft[32 * h : 32 * h + 1, 0:64],
                            rowb[32 * h : 32 * h + 1, :],
                            start=True, stop=True,
                            tile_position=(32 * h, 0),
                        )
                        rb = RBP.tile([64, SBLK], F32, name="rb", tag="rb")
                        nc.vector.tensor_copy(rb[:], ps_rb[:])
                        nc.vector.tensor_tensor(
                            outstk[h // 2][qb : qb + 64, b * SBLK : (b + 1) * SBLK],
                            pso[h][0:64, :],
                            rb[:],
                            op=OP.mult,
                        )
                    # output projection for this block's 4 sq tiles
                    for st in range(4 * b, 4 * b + 4):
                        for db in range(NSB):
                            ds = slice(db * SBLK, (db + 1) * SBLK)
                            psf = AUX.tile([128, SBLK], F32, name="psf", tag="aux")
                            nc.tensor.matmul(
                                psf[:],
                                outstk[0][:, st * 128 : (st + 1) * 128],
                                wo_t[0][:, ds],
                                start=True,
                                stop=False,
                            )
                            nc.tensor.matmul(
                                psf[:],
                                outstk[1][:, st * 128 : (st + 1) * 128],
                                wo_t[1][:, ds],
                                start=False,
                                stop=True,
                            )
                            ot = OE.tile([128, SBLK], F32, name="ot", tag="oe")
                            nc.vector.tensor_tensor(
                                ot[:], psf[:], biasb[:, ds], op=OP.add
                            )
                            nc.sync.dma_start(
                                out[st * 128 : (st + 1) * 128, ds], ot[:]
                            )
                    if b == 0:
                        for t in range(12, 16):
                            pv2 = AUX.tile([128, HD], BF16, name="pv2", tag="aux")
                            nc.tensor.transpose(
                                pv2[:], vT[:, t * 128 : (t + 1) * 128], idp_t[:]
                            )
                            nc.vector.tensor_copy(
                                vext[:, t * 65 : t * 65 + 64], pv2[:]
                            )
                            nc.vector.tensor_copy(
                                vext[:, t * 65 + 64 : t * 65 + 65], onesb_t[:]
                            )

    _fix_range_clears(nc)
    if split_waits:
        _split_excess_waits(nc)
    return nc


_nc_cache = [None]


def kernel(**inputs):
    in_maps = prep_inputs(inputs)
    if _nc_cache[0] is None:
        _nc_cache[0] = build_nc()
    nc = _nc_cache[0]
    res = run_bass_kernel_spmd(nc, in_maps, list(range(NCORES)))
    acc = res.results[0]["out"].astype(np.float32)
    for i in range(1, NCORES):
        acc = acc + res.results[i]["out"]
    return acc.reshape(B, S, DIM)


# revision 40
# speedup vs baseline: 1.0117x; 1.0117x over previous
"""Trainium2 Bass kernel for nn_Attention_4037269258732 (GQA attention with
RoPE, causal mask, and per-head sink-logit LSE renormalization).

Problem:  B=1, S=2048, DIM=2048, H=32 q-heads, KVH=8 kv-heads, HD=64.
          out = Wo @ attn(RoPE(Wq x), RoPE(Wk x), Wv x) + bo, causal,
          with out rows scaled by sigmoid(lse - sink_h).

Sharding (8 cores, tensor-parallel over heads):
  core c owns q-heads [4c, 4c+4), kv-head c, the matching rows of
  wq/wk/wv, wo's input-dim slice [256c, 256c+256), and sinks[4c:4c+4].
  Each core computes a full-shape [S, DIM] partial of the output
  projection (wo_b/8 added on every core); the host sums the 8 partials
  (that sum is the o-dim contraction of the output projection).

Device dataflow per core (feature dims on SBUF partitions so every
matmul chains without transposes; bf16 matmul operands / fp32 PSUM):
  qT[256,S], kT[64,S], vT[64,S] = W.T @ xT      (xT host-transposed)
  RoPE fused into PSUM eviction: q = (q+b)*cos + shifted(q+b)*sin_rot
  (rot_half as 32-partition-shifted DVE reads; sign folded into sin_rot)
  v_nat[S,64] via PE transpose;  Vext = [v_nat | 1]
  per (block b of 512 sq, sk-tile t, head h):
    P^T[sk,sq] = exp(kT_t.T @ q / 8)   (tri-mask on diagonal subtiles,
                                        upper-right tiles skipped)
    outT_ext[65,sq] += Vext_t.T @ P^T  (row 64 = sum_exp = softmax denom)
  per block: renorm rows r=sum_exp+e^sink at partitions {0,32,64,96},
    1/r via exp(-ln r) on ACT, broadcast via K=1 matmul,
    out_norm = outT * bcast;  then final[sq_tile, :] = outstk.T @ woT
    + wo_b/8 -> DRAM partial
"""

import numpy as np
import ml_dtypes

import bass_rust
import concourse.bass as bass
import concourse.tile as tile
from concourse import mybir
from concourse.bass_utils import run_bass_kernel_spmd

F32 = mybir.dt.float32
BF16 = mybir.dt.bfloat16
AF = mybir.ActivationFunctionType
OP = mybir.AluOpType
BF = ml_dtypes.bfloat16

B, S, DIM = 1, 2048, 2048
H, KVH, HD = 32, 8, 64
NCORES = 8
QH = H // NCORES          # 4 q heads per core
SBLK = 512                # sq block size
NSB = S // SBLK           # 4
NDC = DIM // 128          # 16 contraction chunks
NST = S // 128            # 16 sk tiles
SCALE = 1.0 / 8.0         # 1/sqrt(HD)

_ws_ctr = [0]


def _fix_range_clears(nc):
    """walrus here rejects the EVENT_SEMAPHORE_RANGE_CLEAR ISA struct
    ("ISA wrong length"); replace with per-sem write-0 NoOps."""
    import re as _re
    for f in nc.m.functions:
        for blk in f.blocks:
            out, changed = [], False
            for inst in blk.instructions:
                if type(inst).__name__ == "InstISA" and inst.isa_opcode == 176:
                    m = _re.search(r"range_first=(\d+) range_last=(\d+)", inst.concise())
                    first, last = int(m.group(1)), int(m.group(2))
                    for semid in range(first, last + 1):
                        _ws_ctr[0] += 1
                        nop = mybir.InstNoOp(name=f"I-rc-{_ws_ctr[0]}", ins=[], outs=[])
                        nop.engine = inst.engine
                        nop.sync_info = bass_rust.SyncInfo(
                            on_wait=[],
                            on_update=[
                                bass_rust.SyncUpdate(
                                    sync_type="semaphore",
                                    id=semid,
                                    update_mode="sem-wr-imm",
                                    update_value=0,
                                )
                            ],
                        )
                        out.append(nop)
                    changed = True
                    continue
                out.append(inst)
            if changed:
                blk.instructions = out


def _split_excess_waits(nc, max_waits=1):
    """walrus on this image encodes at most one SyncWait per instruction;
    hoist excess waits onto same-engine NoOps placed just before."""
    for f in nc.m.functions:
        for blk in f.blocks:
            out, changed = [], False
            for inst in blk.instructions:
                si = inst.sync_info
                waits = list(si.on_wait) if si is not None else []
                if len(waits) > max_waits:
                    excess, keep = waits[:-max_waits], waits[-max_waits:]
                    for k in range(0, len(excess), max_waits):
                        _ws_ctr[0] += 1
                        nop = mybir.InstNoOp(name=f"I-ws-{_ws_ctr[0]}", ins=[], outs=[])
                        nop.engine = inst.engine
                        nop.sync_info = bass_rust.SyncInfo(
                            on_wait=excess[k : k + max_waits], on_update=[]
                        )
                        out.append(nop)
                    inst.sync_info = bass_rust.SyncInfo(
                        on_wait=keep, on_update=list(si.on_update)
                    )
                    changed = True
                out.append(inst)
            if changed:
                blk.instructions = out


def prep_inputs(inputs):
    """Host-side sharding/layout prep. Returns per-core input maps."""
    x = np.asarray(inputs["x"], np.float32)
    rope = np.asarray(inputs["rope_cache"], np.float32)
    wq = np.asarray(inputs["wq_w"], np.float32)
    bq = np.asarray(inputs["wq_b"], np.float32)
    wk = np.asarray(inputs["wk_w"], np.float32)
    bk = np.asarray(inputs["wk_b"], np.float32)
    wv = np.asarray(inputs["wv_w"], np.float32)
    bv = np.asarray(inputs["wv_b"], np.float32)
    wo = np.asarray(inputs["wo_w"], np.float32)
    bo = np.asarray(inputs["wo_b"], np.float32)
    sinks = np.asarray(inputs["sinks"], np.float32)

    xT = np.ascontiguousarray(x[0].T).astype(BF)            # [DIM, S]
    cosT = rope[:, :HD].T                                   # [64, S]
    sinT = rope[:, HD:].T
    cos2 = np.ascontiguousarray(np.concatenate([cosT, cosT], 0)).astype(np.float32)
    # sin_rot indexed by SOURCE partition: source rows hd in [0,32) land at
    # out rows hd+32 with +sin[hd+32]; source rows hd in [32,64) land at
    # out rows hd-32 with -sin[hd-32]. Duplicated for both heads per tile.
    sr = np.concatenate([sinT[32:64], -sinT[0:32]], 0)      # [64, S]
    sin_rot2 = np.ascontiguousarray(np.concatenate([sr, sr], 0)).astype(np.float32)
    tri = np.triu(np.ones((128, 128), BF))                  # mask[p, j] = j >= p
    ident = np.eye(HD, dtype=BF)
    wob8 = (bo / NCORES).reshape(1, DIM).astype(np.float32)

    in_maps = []
    for c in range(NCORES):
        qs = slice(c * QH * HD, (c + 1) * QH * HD)          # 256 q rows
        ks = slice(c * HD, (c + 1) * HD)                    # 64 kv rows
        # wproj columns: [q 256 | k 64 | v 64] = 384
        wproj = np.concatenate([wq[qs].T, wk[ks].T, wv[ks].T], axis=1)
        bcol = np.zeros((128, 3), np.float32)
        bcol[:, 0] = bq[qs][0:128]
        bcol[:, 1] = bq[qs][128:256]
        bcol[0:64, 2] = bk[ks]
        bcol[64:128, 2] = bv[ks]
        woT = np.ascontiguousarray(wo[:, qs].T).astype(BF)  # [256, DIM]
        esink = np.tile(np.exp(sinks[c * QH : (c + 1) * QH]).reshape(1, QH),
                        (128, 1))
        in_maps.append(
            {
                "xT": xT,
                "wproj": np.ascontiguousarray(wproj).astype(BF),
                "bproj": bcol,
                "cos2": cos2,
                "sinr2": sin_rot2,
                "woT": woT,
                "wob8": wob8,
                "esink": esink.astype(np.float32),
                "tri": tri,
                "ident": ident,
                "ones_f": np.ones((128, 128), np.float32),
                "onesb": np.ones((128, 1), BF),
            }
        )
    return in_maps


def build_nc(split_waits=True):
    nc = bass.Bass("TRN2", target_bir_lowering=False, debug=False, num_devices=NCORES)
    xT = nc.dram_tensor("xT", [DIM, S], BF16, kind="ExternalInput").ap()
    wproj = nc.dram_tensor("wproj", [DIM, 384], BF16, kind="ExternalInput").ap()
    bproj = nc.dram_tensor("bproj", [128, 3], F32, kind="ExternalInput").ap()
    cos2 = nc.dram_tensor("cos2", [128, S], F32, kind="ExternalInput").ap()
    sinr2 = nc.dram_tensor("sinr2", [128, S], F32, kind="ExternalInput").ap()
    woT = nc.dram_tensor("woT", [2 * 128, DIM], BF16, kind="ExternalInput").ap()
    wob8 = nc.dram_tensor("wob8", [1, DIM], F32, kind="ExternalInput").ap()
    esink = nc.dram_tensor("esink", [128, QH], F32, kind="ExternalInput").ap()
    tri = nc.dram_tensor("tri", [128, 128], BF16, kind="ExternalInput").ap()
    ident = nc.dram_tensor("ident", [HD, HD], BF16, kind="ExternalInput").ap()
    ones_f = nc.dram_tensor("ones_f", [128, 128], F32, kind="ExternalInput").ap()
    onesb = nc.dram_tensor("onesb", [128, 1], BF16, kind="ExternalInput").ap()
    out = nc.dram_tensor("out", [S, DIM], F32, kind="ExternalOutput").ap()

    with tile.TileContext(nc) as tc:
        with tc.tile_pool(name="persist", bufs=1) as P:
            # ---- long-lived tiles ----
            esink_t = P.tile([128, QH], F32, tag="esink")
            tri_t = P.tile([128, 128], BF16, tag="tri")
            wo_t = [
                P.tile([128, DIM], BF16, name=f"wo{i}", tag=f"wo{i}")
                for i in range(2)
            ]
            biasb = P.tile([128, DIM], F32, tag="biasb")
            ones_ft = P.tile([128, 128], F32, tag="ones_ft")
            wob_row = P.tile([1, DIM], F32, tag="wobrow")
            # tiny dummy Exp/Ln to pull the ACT table load off the
            # attention critical path
            scr = P.tile([1, 16], F32, tag="scr")
            qp = [P.tile([128, S], BF16, name=f"qp{i}", tag=f"qp{i}") for i in range(2)]
            kT2 = P.tile([128, S], BF16, tag="kT2")
            vext = P.tile([128, NST * (HD + 1)], BF16, tag="vext")
            onesb_t = P.tile([128, 1], BF16, tag="onesb_t")
            outstk = [P.tile([128, S], BF16, name=f"os{i}", tag=f"os{i}") for i in range(2)]
            vT = P.tile([64, S], BF16, tag="vT")
            idp_t = P.tile([HD, HD], BF16, tag="idp")

            # ---- qkv projection, rope fused into eviction ----
            with (
                tc.tile_pool(name="projw", bufs=1) as PW,
                tc.tile_pool(name="tmp", bufs=2) as TMP,
                tc.tile_pool(name="psproj", bufs=2, space="PSUM") as PSP,
                tc.tile_pool(name="psv", bufs=2, space="PSUM") as PSV,
            ):
                x_t, w_t = [], []
                for dc in range(NDC):
                    wt = PW.tile([128, 384], BF16, name=f"w{dc}", tag=f"w{dc}")
                    nc.gpsimd.dma_start(wt[:], wproj[dc * 128 : (dc + 1) * 128, :])
                    w_t.append(wt)
                    xt = PW.tile([128, S], BF16, name=f"x{dc}", tag=f"x{dc}")
                    # alternate the two DMA rings so x supply is not limited
                    # by a single HWDGE ring's sustained rate
                    xeng = nc.sync if dc % 2 == 0 else nc.gpsimd
                    if dc < 2:
                        for q4 in range(4):
                            xeng.dma_start(
                                xt[:, q4 * SBLK : (q4 + 1) * SBLK],
                                xT[dc * 128 : (dc + 1) * 128,
                                   q4 * SBLK : (q4 + 1) * SBLK],
                            )
                    else:
                        xeng.dma_start(xt[:], xT[dc * 128 : (dc + 1) * 128, :])
                    x_t.append(xt)
                bcol_t = PW.tile([128, 3], F32, tag="bcol")
                nc.gpsimd.dma_start(bcol_t[:], bproj[:])
                cos_t = PW.tile([128, S], F32, tag="cos")
                nc.gpsimd.dma_start(cos_t[:], cos2[:])
                sinr_t = PW.tile([128, S], F32, tag="sinr")
                nc.gpsimd.dma_start(sinr_t[:], sinr2[:])
                id_t = PW.tile([HD, HD], BF16, tag="ident")
                nc.gpsimd.dma_start(id_t[:], ident[:])
                nc.gpsimd.dma_start(onesb_t[:], onesb[:])
                nc.gpsimd.dma_start(esink_t[:], esink[:])
                nc.gpsimd.dma_start(tri_t[:], tri[:])
                nc.gpsimd.dma_start(ones_ft[:], ones_f[:])
                nc.gpsimd.dma_start(wob_row[:], wob8[:])
                for i in range(2):
                    nc.gpsimd.dma_start(
                        wo_t[i][:], woT[i * 128 : (i + 1) * 128, :]
                    )
                nc.gpsimd.dma_start(idp_t[:], ident[:])
                nc.scalar.activation(scr[0:1, 0:3], bcol_t[0:1, 0:3], AF.Exp)
                nc.scalar.activation(scr[0:1, 0:3], scr[0:1, 0:3], AF.Ln)

                for sb in range(NSB):
                    ss = slice(sb * SBLK, (sb + 1) * SBLK)
                    ps = [
                        PSP.tile([128, SBLK], F32, name=f"pp{j}", tag=f"pp{j}")
                        for j in range(3)
                    ]
                    for dc in range(NDC):
                        for j, (c0, c1) in enumerate(
                            [(0, 128), (128, 256), (256, 384)]
                        ):
                            nc.tensor.matmul(
                                ps[j][:],
                                w_t[dc][:, c0:c1],
                                x_t[dc][:, ss],
                                start=(dc == 0),
                                stop=(dc == NDC - 1),
                            )
                    # rope eviction: cos part for both heads of a ptile at
                    # once; rot part via 32-partition-shifted reads with the
                    # sign folded into sinr_t; combine per head into qh (bf16)
                    for i in range(2):
                        t1 = TMP.tile([128, SBLK], F32, name="t1", tag="t1")
                        nc.vector.scalar_tensor_tensor(
                            t1[:], ps[i][:], bcol_t[:, i : i + 1], cos_t[:, ss],
                            op0=OP.add, op1=OP.mult,
                        )
                        t2 = TMP.tile([128, SBLK], F32, name="t2", tag="t2")
                        for g in range(4):
                            d0 = 32 * g
                            s0 = 32 * g + 32 if g % 2 == 0 else 32 * g - 32
                            nc.vector.scalar_tensor_tensor(
                                t2[d0 : d0 + 32, :],
                                ps[i][s0 : s0 + 32, :],
                                bcol_t[s0 : s0 + 32, i : i + 1],
                                sinr_t[s0 : s0 + 32, ss],
                                op0=OP.add, op1=OP.mult,
                            )
                        nc.vector.tensor_tensor(
                            qp[i][:, ss], t1[:], t2[:], op=OP.add
                        )
                    # k: rows 0:64 of ps[2]
                    tk1 = TMP.tile([64, SBLK], F32, name="tk1", tag="tk1")
                    nc.vector.scalar_tensor_tensor(
                        tk1[:], ps[2][0:64, :], bcol_t[0:64, 2:3], cos_t[0:64, ss],
                        op0=OP.add, op1=OP.mult,
                    )
                    tk2 = TMP.tile([64, SBLK], F32, name="tk2", tag="tk2")
                    nc.vector.scalar_tensor_tensor(
                        tk2[0:32, :], ps[2][32:64, :], bcol_t[32:64, 2:3],
                        sinr_t[32:64, ss], op0=OP.add, op1=OP.mult,
                    )
                    nc.vector.scalar_tensor_tensor(
                        tk2[32:64, :], ps[2][0:32, :], bcol_t[0:32, 2:3],
                        sinr_t[0:32, ss], op0=OP.add, op1=OP.mult,
                    )
                    nc.vector.tensor_tensor(
                        kT2[0:64, ss], tk1[:], tk2[:], op=OP.add
                    )
                    nc.vector.tensor_copy(kT2[64:128, ss], kT2[0:64, ss])
                    # v: rows 64:128 of ps[2], bias only
                    nc.vector.tensor_scalar_add(
                        vT[:, ss], ps[2][64:128, :], bcol_t[64:128, 2:3]
                    )
                    # transpose this block's v tiles into Vext right away
                    # (sb=3's transposes are deferred past attention block 0
                    # so the PE doesn't stall on the last rope eviction)
                    if sb < 3:
                        for t in range(4 * sb, 4 * sb + 4):
                            pv = PSV.tile([128, HD], BF16, name="pv", tag="pv")
                            nc.tensor.transpose(
                                pv[:], vT[:, t * 128 : (t + 1) * 128], id_t[:]
                            )
                            nc.vector.tensor_copy(
                                vext[:, t * 65 : t * 65 + 64], pv[:]
                            )
                            nc.vector.tensor_copy(
                                vext[:, t * 65 + 64 : t * 65 + 65], onesb_t[:]
                            )

            # ---- attention + per-block renorm + output projection ----
            with (
                tc.tile_pool(name="pss", bufs=2, space="PSUM") as PSS,
                tc.tile_pool(name="pso", bufs=1, space="PSUM") as PSO,
                tc.tile_pool(name="aux", bufs=2, space="PSUM") as AUX,
                tc.tile_pool(name="ptp", bufs=8) as PTP,
                tc.tile_pool(name="rows", bufs=2) as RP,
                tc.tile_pool(name="rbp", bufs=2) as RBP,
                tc.tile_pool(name="oev", bufs=4) as OE,
            ):
                # wo bias broadcast rows (K=1 matmuls)
                for db in range(NSB):
                    ds = slice(db * SBLK, (db + 1) * SBLK)
                    ps_bb = AUX.tile([128, SBLK], F32, name="ps_bb", tag="aux")
                    nc.tensor.matmul(
                        ps_bb[:], ones_ft[0:1, :], wob_row[0:1, ds],
                        start=True, stop=True,
                    )
                    nc.vector.tensor_copy(biasb[:, ds], ps_bb[:])
                for b in range(NSB):
                    pso = [
                        PSO.tile([65, SBLK], F32, name=f"oo{i}", tag=f"oo{i}")
                        for i in range(QH)
                    ]
                    nt = 4 * b + 4
                    for t in range(nt):
                        off = 128 * (t - 4 * b) if t >= 4 * b else 0
                        ptts = []
                        for hp in range(2):
                            # two K=64 score matmuls packed into disjoint
                            # PE row groups -> run concurrently
                            psa = PSS.tile([128, SBLK], F32, name="psa", tag="ss")
                            psb = PSS.tile([128, SBLK], F32, name="psb", tag="ss")
                            nc.tensor.matmul(
                                psa[:, off:SBLK],
                                kT2[0:64, t * 128 : (t + 1) * 128],
                                qp[hp][0:64, b * SBLK + off : (b + 1) * SBLK],
                                start=True,
                                stop=True,
                                tile_position=(0, 0),
                            )
                            nc.tensor.matmul(
                                psb[:, off:SBLK],
                                kT2[64:128, t * 128 : (t + 1) * 128],
                                qp[hp][64:128, b * SBLK + off : (b + 1) * SBLK],
                                start=True,
                                stop=True,
                                tile_position=(64, 0),
                            )
                            for lane, pss in ((0, psa), (1, psb)):
                                ptt = PTP.tile([128, SBLK], BF16, name="ptt", tag="pt")
                                nc.scalar.activation(
                                    ptt[:, off:SBLK], pss[:, off:SBLK], AF.Exp,
                                    scale=SCALE,
                                )
                                if t >= 4 * b:
                                    nc.vector.tensor_tensor(
                                        ptt[:, off : off + 128],
                                        ptt[:, off : off + 128],
                                        tri_t[:],
                                        op=OP.mult,
                                    )
                                ptts.append(ptt)
                        for h in range(QH):
                            nc.tensor.matmul(
                                pso[h][:, off:SBLK],
                                vext[:, t * 65 : (t + 1) * 65],
                                ptts[h][:, off:SBLK],
                                start=(t == 0),
                                stop=(t == nt - 1),
                            )
                    # sink renorm via exp(-ln r) + K=1 bcast matmul
                    rowb = RP.tile([128, SBLK], F32, name="rowb", tag="rowb")
                    nc.gpsimd.memset(rowb[:], 1.0)
                    for h in range(QH):
                        nc.vector.tensor_scalar_add(
                            rowb[32 * h : 32 * h + 1, :],
                            pso[h][64:65, :],
                            esink_t[64:65, h : h + 1],
                        )
                    rinvb = RP.tile([128, SBLK], F32, name="rinvb", tag="rinvb")
                    nc.scalar.activation(rinvb[:], rowb[:], AF.Ln)
                    nc.scalar.activation(rowb[:], rinvb[:], AF.Exp, scale=-1.0)
                    for h in range(QH):
                        qb = (h % 2) * 64
                        ps_rb = AUX.tile([64, SBLK], F32, name="ps_rb", tag="aux")
                        nc.tensor.matmul(
                            ps_rb[:], ones_ft[32 * h : 32 * h + 1, 0:64],
                            rowb[32 * h : 32 * h + 1, :],
                            start=True, stop=True,
                            tile_position=(32 * h, 0),
                        )
                        rb = RBP.tile([64, SBLK], F32, name="rb", tag="rb")
                        nc.vector.tensor_copy(rb[:], ps_rb[:])
                        nc.vector.tensor_tensor(
                            outstk[h // 2][qb : qb + 64, b * SBLK : (b + 1) * SBLK],
                            pso[h][0:64, :],
                            rb[:],
                            op=OP.mult,
                        )
                    # output projection for this block's 4 sq tiles
                    for st in range(4 * b, 4 * b + 4):
                        for db in range(NSB):
                            ds = slice(db * SBLK, (db + 1) * SBLK)
                            psf = AUX.tile([128, SBLK], F32, name="psf", tag="aux")
                            nc.tensor.matmul(
                                psf[:],
                                outstk[0][:, st * 128 : (st + 1) * 128],
                                wo_t[0][:, ds],
                                start=True,
                                stop=False,
                            )
                            nc.tensor.matmul(
                                psf[:],
                                outstk[1][:, st * 128 : (st + 1) * 128],
                                wo_t[1][:, ds],
                                start=False,
                                stop=True,
                            )
                            ot = OE.tile([128, SBLK], F32, name="ot", tag="oe")
                            nc.vector.tensor_tensor(
                                ot[:], psf[:], biasb[:, ds], op=OP.add
                            )
                            nc.sync.dma_start(
                                out[st * 128 : (st + 1) * 128, ds], ot[:]
                            )
                    if b == 0:
                        for t in range(12, 16):
                            pv2 = AUX.tile([128, HD], BF16, name="pv2", tag="aux")
                            nc.tensor.transpose(
                                pv2[:], vT[:, t * 128 : (t + 1) * 128], idp_t[:]
                            )
                            nc.vector.tensor_copy(
                                vext[:, t * 65 : t * 65 + 64], pv2[:]
                            )
                            nc.vector.tensor_copy(
                                vext[:, t * 65 + 64 : t * 65 + 65], onesb_t[:]
                            )

    _fix_range_clears(nc)
    if split_waits:
        _split_excess_waits(nc)
    return nc


_nc_cache = [None]


def kernel(**inputs):
    in_maps = prep_inputs(inputs)
    if _nc_cache[0] is None:
        _nc_cache[0] = build_nc()
    nc = _nc_cache[0]
    res = run_bass_kernel_spmd(nc, in_maps, list(range(NCORES)))
    acc = res.results[0]["out"].astype(np.float32)
    for i in range(1, NCORES):
        acc = acc + res.results[i]["out"]
    return acc.reshape(B, S, DIM)


# revision 41
# speedup vs baseline: 1.0189x; 1.0072x over previous
"""Trainium2 Bass kernel for nn_Attention_4037269258732 (GQA attention with
RoPE, causal mask, and per-head sink-logit LSE renormalization).

Problem:  B=1, S=2048, DIM=2048, H=32 q-heads, KVH=8 kv-heads, HD=64.
          out = Wo @ attn(RoPE(Wq x), RoPE(Wk x), Wv x) + bo, causal,
          with out rows scaled by sigmoid(lse - sink_h).

Sharding (8 cores, tensor-parallel over heads):
  core c owns q-heads [4c, 4c+4), kv-head c, the matching rows of
  wq/wk/wv, wo's input-dim slice [256c, 256c+256), and sinks[4c:4c+4].
  Each core computes a full-shape [S, DIM] partial of the output
  projection (wo_b/8 added on every core); the host sums the 8 partials
  (that sum is the o-dim contraction of the output projection).

Device dataflow per core (feature dims on SBUF partitions so every
matmul chains without transposes; bf16 matmul operands / fp32 PSUM):
  qT[256,S], kT[64,S], vT[64,S] = W.T @ xT      (xT host-transposed)
  RoPE fused into PSUM eviction: q = (q+b)*cos + shifted(q+b)*sin_rot
  (rot_half as 32-partition-shifted DVE reads; sign folded into sin_rot)
  v_nat[S,64] via PE transpose;  Vext = [v_nat | 1]
  per (block b of 512 sq, sk-tile t, head h):
    P^T[sk,sq] = exp(kT_t.T @ q / 8)   (tri-mask on diagonal subtiles,
                                        upper-right tiles skipped)
    outT_ext[65,sq] += Vext_t.T @ P^T  (row 64 = sum_exp = softmax denom)
  per block: renorm rows r=sum_exp+e^sink at partitions {0,32,64,96},
    1/r via exp(-ln r) on ACT, broadcast via K=1 matmul,
    out_norm = outT * bcast;  then final[sq_tile, :] = outstk.T @ woT
    + wo_b/8 -> DRAM partial
"""

import numpy as np
import ml_dtypes

import bass_rust
import concourse.bass as bass
import concourse.tile as tile
from concourse import mybir
from concourse.bass_utils import run_bass_kernel_spmd

F32 = mybir.dt.float32
BF16 = mybir.dt.bfloat16
AF = mybir.ActivationFunctionType
OP = mybir.AluOpType
BF = ml_dtypes.bfloat16

B, S, DIM = 1, 2048, 2048
H, KVH, HD = 32, 8, 64
NCORES = 8
QH = H // NCORES          # 4 q heads per core
SBLK = 512                # sq block size
NSB = S // SBLK           # 4
NDC = DIM // 128          # 16 contraction chunks
NST = S // 128            # 16 sk tiles
SCALE = 1.0 / 8.0         # 1/sqrt(HD)

_ws_ctr = [0]


def _fix_range_clears(nc):
    """walrus here rejects the EVENT_SEMAPHORE_RANGE_CLEAR ISA struct
    ("ISA wrong length"); replace with per-sem write-0 NoOps."""
    import re as _re
    for f in nc.m.functions:
        for blk in f.blocks:
            out, changed = [], False
            for inst in blk.instructions:
                if type(inst).__name__ == "InstISA" and inst.isa_opcode == 176:
                    m = _re.search(r"range_first=(\d+) range_last=(\d+)", inst.concise())
                    first, last = int(m.group(1)), int(m.group(2))
                    for semid in range(first, last + 1):
                        _ws_ctr[0] += 1
                        nop = mybir.InstNoOp(name=f"I-rc-{_ws_ctr[0]}", ins=[], outs=[])
                        nop.engine = inst.engine
                        nop.sync_info = bass_rust.SyncInfo(
                            on_wait=[],
                            on_update=[
                                bass_rust.SyncUpdate(
                                    sync_type="semaphore",
                                    id=semid,
                                    update_mode="sem-wr-imm",
                                    update_value=0,
                                )
                            ],
                        )
                        out.append(nop)
                    changed = True
                    continue
                out.append(inst)
            if changed:
                blk.instructions = out


def _split_excess_waits(nc, max_waits=1):
    """walrus on this image encodes at most one SyncWait per instruction;
    hoist excess waits onto same-engine NoOps placed just before."""
    for f in nc.m.functions:
        for blk in f.blocks:
            out, changed = [], False
            for inst in blk.instructions:
                si = inst.sync_info
                waits = list(si.on_wait) if si is not None else []
                if len(waits) > max_waits:
                    excess, keep = waits[:-max_waits], waits[-max_waits:]
                    for k in range(0, len(excess), max_waits):
                        _ws_ctr[0] += 1
                        nop = mybir.InstNoOp(name=f"I-ws-{_ws_ctr[0]}", ins=[], outs=[])
                        nop.engine = inst.engine
                        nop.sync_info = bass_rust.SyncInfo(
                            on_wait=excess[k : k + max_waits], on_update=[]
                        )
                        out.append(nop)
                    inst.sync_info = bass_rust.SyncInfo(
                        on_wait=keep, on_update=list(si.on_update)
                    )
                    changed = True
                out.append(inst)
            if changed:
                blk.instructions = out


def prep_inputs(inputs):
    """Host-side sharding/layout prep. Returns per-core input maps."""
    x = np.asarray(inputs["x"], np.float32)
    rope = np.asarray(inputs["rope_cache"], np.float32)
    wq = np.asarray(inputs["wq_w"], np.float32)
    bq = np.asarray(inputs["wq_b"], np.float32)
    wk = np.asarray(inputs["wk_w"], np.float32)
    bk = np.asarray(inputs["wk_b"], np.float32)
    wv = np.asarray(inputs["wv_w"], np.float32)
    bv = np.asarray(inputs["wv_b"], np.float32)
    wo = np.asarray(inputs["wo_w"], np.float32)
    bo = np.asarray(inputs["wo_b"], np.float32)
    sinks = np.asarray(inputs["sinks"], np.float32)

    xT = np.ascontiguousarray(x[0].T).astype(BF)            # [DIM, S]
    cosT = rope[:, :HD].T                                   # [64, S]
    sinT = rope[:, HD:].T
    cos2 = np.ascontiguousarray(np.concatenate([cosT, cosT], 0)).astype(np.float32)
    # sin_rot indexed by SOURCE partition: source rows hd in [0,32) land at
    # out rows hd+32 with +sin[hd+32]; source rows hd in [32,64) land at
    # out rows hd-32 with -sin[hd-32]. Duplicated for both heads per tile.
    sr = np.concatenate([sinT[32:64], -sinT[0:32]], 0)      # [64, S]
    sin_rot2 = np.ascontiguousarray(np.concatenate([sr, sr], 0)).astype(np.float32)
    tri = np.triu(np.ones((128, 128), BF))                  # mask[p, j] = j >= p
    ident = np.eye(HD, dtype=BF)
    wob8 = (bo / NCORES).reshape(1, DIM).astype(np.float32)

    in_maps = []
    for c in range(NCORES):
        qs = slice(c * QH * HD, (c + 1) * QH * HD)          # 256 q rows
        ks = slice(c * HD, (c + 1) * HD)                    # 64 kv rows
        # wproj columns: [q 256 | k 64 | v 64] = 384
        wproj = np.concatenate([wq[qs].T, wk[ks].T, wv[ks].T], axis=1)
        bcol = np.zeros((128, 3), np.float32)
        bcol[:, 0] = bq[qs][0:128]
        bcol[:, 1] = bq[qs][128:256]
        bcol[0:64, 2] = bk[ks]
        bcol[64:128, 2] = bv[ks]
        woT = np.ascontiguousarray(wo[:, qs].T).astype(BF)  # [256, DIM]
        esink = np.tile(np.exp(sinks[c * QH : (c + 1) * QH]).reshape(1, QH),
                        (128, 1))
        in_maps.append(
            {
                "xT": xT,
                "wproj": np.ascontiguousarray(wproj).astype(BF),
                "bproj": bcol,
                "cos2": cos2,
                "sinr2": sin_rot2,
                "woT": woT,
                "wob8": wob8,
                "esink": esink.astype(np.float32),
                "tri": tri,
                "ident": ident,
                "ones_f": np.ones((128, 128), np.float32),
                "onesb": np.ones((128, 1), BF),
            }
        )
    return in_maps


def build_nc(split_waits=True):
    nc = bass.Bass("TRN2", target_bir_lowering=False, debug=False, num_devices=NCORES)
    xT = nc.dram_tensor("xT", [DIM, S], BF16, kind="ExternalInput").ap()
    wproj = nc.dram_tensor("wproj", [DIM, 384], BF16, kind="ExternalInput").ap()
    bproj = nc.dram_tensor("bproj", [128, 3], F32, kind="ExternalInput").ap()
    cos2 = nc.dram_tensor("cos2", [128, S], F32, kind="ExternalInput").ap()
    sinr2 = nc.dram_tensor("sinr2", [128, S], F32, kind="ExternalInput").ap()
    woT = nc.dram_tensor("woT", [2 * 128, DIM], BF16, kind="ExternalInput").ap()
    wob8 = nc.dram_tensor("wob8", [1, DIM], F32, kind="ExternalInput").ap()
    esink = nc.dram_tensor("esink", [128, QH], F32, kind="ExternalInput").ap()
    tri = nc.dram_tensor("tri", [128, 128], BF16, kind="ExternalInput").ap()
    ident = nc.dram_tensor("ident", [HD, HD], BF16, kind="ExternalInput").ap()
    ones_f = nc.dram_tensor("ones_f", [128, 128], F32, kind="ExternalInput").ap()
    onesb = nc.dram_tensor("onesb", [128, 1], BF16, kind="ExternalInput").ap()
    out = nc.dram_tensor("out", [S, DIM], F32, kind="ExternalOutput").ap()

    with tile.TileContext(nc) as tc:
        with tc.tile_pool(name="persist", bufs=1) as P:
            # ---- long-lived tiles ----
            esink_t = P.tile([128, QH], F32, tag="esink")
            tri_t = P.tile([128, 128], BF16, tag="tri")
            wo_t = [
                P.tile([128, DIM], BF16, name=f"wo{i}", tag=f"wo{i}")
                for i in range(2)
            ]
            biasb = P.tile([128, DIM], F32, tag="biasb")
            ones_ft = P.tile([128, 128], F32, tag="ones_ft")
            wob_row = P.tile([1, DIM], F32, tag="wobrow")
            # tiny dummy Exp/Ln to pull the ACT table load off the
            # attention critical path
            scr = P.tile([1, 16], F32, tag="scr")
            qp = [P.tile([128, S], BF16, name=f"qp{i}", tag=f"qp{i}") for i in range(2)]
            kT2 = P.tile([128, S], BF16, tag="kT2")
            vext = P.tile([128, NST * (HD + 1)], BF16, tag="vext")
            onesb_t = P.tile([128, 1], BF16, tag="onesb_t")
            outstk = [P.tile([128, S], BF16, name=f"os{i}", tag=f"os{i}") for i in range(2)]
            vT = P.tile([64, S], BF16, tag="vT")
            idp_t = P.tile([HD, HD], BF16, tag="idp")

            # ---- qkv projection, rope fused into eviction ----
            with (
                tc.tile_pool(name="projw", bufs=1) as PW,
                tc.tile_pool(name="tmp", bufs=2) as TMP,
                tc.tile_pool(name="psproj", bufs=2, space="PSUM") as PSP,
                tc.tile_pool(name="psv", bufs=2, space="PSUM") as PSV,
            ):
                x_t, w_t = [], []
                for dc in range(NDC):
                    wt = PW.tile([128, 384], BF16, name=f"w{dc}", tag=f"w{dc}")
                    nc.gpsimd.dma_start(wt[:], wproj[dc * 128 : (dc + 1) * 128, :])
                    w_t.append(wt)
                    xt = PW.tile([128, S], BF16, name=f"x{dc}", tag=f"x{dc}")
                    if dc < 2:
                        for q4 in range(4):
                            nc.sync.dma_start(
                                xt[:, q4 * SBLK : (q4 + 1) * SBLK],
                                xT[dc * 128 : (dc + 1) * 128,
                                   q4 * SBLK : (q4 + 1) * SBLK],
                            )
                    else:
                        nc.sync.dma_start(xt[:], xT[dc * 128 : (dc + 1) * 128, :])
                    x_t.append(xt)
                bcol_t = PW.tile([128, 3], F32, tag="bcol")
                nc.gpsimd.dma_start(bcol_t[:], bproj[:])
                cos_t = PW.tile([128, S], F32, tag="cos")
                nc.gpsimd.dma_start(cos_t[:], cos2[:])
                sinr_t = PW.tile([128, S], F32, tag="sinr")
                nc.gpsimd.dma_start(sinr_t[:], sinr2[:])
                id_t = PW.tile([HD, HD], BF16, tag="ident")
                nc.gpsimd.dma_start(id_t[:], ident[:])
                nc.gpsimd.dma_start(onesb_t[:], onesb[:])
                nc.gpsimd.dma_start(esink_t[:], esink[:])
                nc.gpsimd.dma_start(tri_t[:], tri[:])
                nc.gpsimd.dma_start(ones_ft[:], ones_f[:])
                nc.gpsimd.dma_start(wob_row[:], wob8[:])
                for i in range(2):
                    nc.gpsimd.dma_start(
                        wo_t[i][:], woT[i * 128 : (i + 1) * 128, :]
                    )
                nc.gpsimd.dma_start(idp_t[:], ident[:])
                nc.scalar.activation(scr[0:1, 0:3], bcol_t[0:1, 0:3], AF.Exp)
                nc.scalar.activation(scr[0:1, 0:3], scr[0:1, 0:3], AF.Ln)

                for sb in range(NSB):
                    ss = slice(sb * SBLK, (sb + 1) * SBLK)
                    ps = [
                        PSP.tile([128, SBLK], F32, name=f"pp{j}", tag=f"pp{j}")
                        for j in range(3)
                    ]
                    for dc in range(NDC):
                        for j, (c0, c1) in enumerate(
                            [(0, 128), (128, 256), (256, 384)]
                        ):
                            nc.tensor.matmul(
                                ps[j][:],
                                w_t[dc][:, c0:c1],
                                x_t[dc][:, ss],
                                start=(dc == 0),
                                stop=(dc == NDC - 1),
                            )
                    # rope eviction: cos part for both heads of a ptile at
                    # once; rot part via 32-partition-shifted reads with the
                    # sign folded into sinr_t; combine per head into qh (bf16)
                    for i in range(2):
                        t1 = TMP.tile([128, SBLK], F32, name="t1", tag="t1")
                        nc.vector.scalar_tensor_tensor(
                            t1[:], ps[i][:], bcol_t[:, i : i + 1], cos_t[:, ss],
                            op0=OP.add, op1=OP.mult,
                        )
                        t2 = TMP.tile([128, SBLK], F32, name="t2", tag="t2")
                        for g in range(4):
                            d0 = 32 * g
                            s0 = 32 * g + 32 if g % 2 == 0 else 32 * g - 32
                            nc.vector.scalar_tensor_tensor(
                                t2[d0 : d0 + 32, :],
                                ps[i][s0 : s0 + 32, :],
                                bcol_t[s0 : s0 + 32, i : i + 1],
                                sinr_t[s0 : s0 + 32, ss],
                                op0=OP.add, op1=OP.mult,
                            )
                        nc.vector.tensor_tensor(
                            qp[i][:, ss], t1[:], t2[:], op=OP.add
                        )
                    # k: rows 0:64 of ps[2]
                    tk1 = TMP.tile([64, SBLK], F32, name="tk1", tag="tk1")
                    nc.vector.scalar_tensor_tensor(
                        tk1[:], ps[2][0:64, :], bcol_t[0:64, 2:3], cos_t[0:64, ss],
                        op0=OP.add, op1=OP.mult,
                    )
                    tk2 = TMP.tile([64, SBLK], F32, name="tk2", tag="tk2")
                    nc.vector.scalar_tensor_tensor(
                        tk2[0:32, :], ps[2][32:64, :], bcol_t[32:64, 2:3],
                        sinr_t[32:64, ss], op0=OP.add, op1=OP.mult,
                    )
                    nc.vector.scalar_tensor_tensor(
                        tk2[32:64, :], ps[2][0:32, :], bcol_t[0:32, 2:3],
                        sinr_t[0:32, ss], op0=OP.add, op1=OP.mult,
                    )
                    nc.vector.tensor_tensor(
                        kT2[0:64, ss], tk1[:], tk2[:], op=OP.add
                    )
                    nc.vector.tensor_copy(kT2[64:128, ss], kT2[0:64, ss])
                    # v: rows 64:128 of ps[2], bias only
                    nc.vector.tensor_scalar_add(
                        vT[:, ss], ps[2][64:128, :], bcol_t[64:128, 2:3]
                    )
                    # transpose this block's v tiles into Vext right away
                    # (sb=3's transposes are deferred past attention block 0
                    # so the PE doesn't stall on the last rope eviction)
                    if sb < 3:
                        for t in range(4 * sb, 4 * sb + 4):
                            pv = PSV.tile([128, HD], BF16, name="pv", tag="pv")
                            nc.tensor.transpose(
                                pv[:], vT[:, t * 128 : (t + 1) * 128], id_t[:]
                            )
                            nc.vector.tensor_copy(
                                vext[:, t * 65 : t * 65 + 64], pv[:]
                            )
                            nc.vector.tensor_copy(
                                vext[:, t * 65 + 64 : t * 65 + 65], onesb_t[:]
                            )

            # ---- attention + per-block renorm + output projection ----
            with (
                tc.tile_pool(name="pss", bufs=2, space="PSUM") as PSS,
                tc.tile_pool(name="pso", bufs=1, space="PSUM") as PSO,
                tc.tile_pool(name="aux", bufs=2, space="PSUM") as AUX,
                tc.tile_pool(name="ptp", bufs=8) as PTP,
                tc.tile_pool(name="rows", bufs=2) as RP,
                tc.tile_pool(name="rbp", bufs=2) as RBP,
                tc.tile_pool(name="oev", bufs=4) as OE,
            ):
                # wo bias broadcast rows (K=1 matmuls)
                for db in range(NSB):
                    ds = slice(db * SBLK, (db + 1) * SBLK)
                    ps_bb = AUX.tile([128, SBLK], F32, name="ps_bb", tag="aux")
                    nc.tensor.matmul(
                        ps_bb[:], ones_ft[0:1, :], wob_row[0:1, ds],
                        start=True, stop=True,
                    )
                    nc.vector.tensor_copy(biasb[:, ds], ps_bb[:])
                for b in range(NSB):
                    pso = [
                        PSO.tile([65, SBLK], F32, name=f"oo{i}", tag=f"oo{i}")
                        for i in range(QH)
                    ]
                    nt = 4 * b + 4
                    for t in range(nt):
                        off = 128 * (t - 4 * b) if t >= 4 * b else 0
                        ptts = []
                        for hp in range(2):
                            # two K=64 score matmuls packed into disjoint
                            # PE row groups -> run concurrently
                            psa = PSS.tile([128, SBLK], F32, name="psa", tag="ss")
                            psb = PSS.tile([128, SBLK], F32, name="psb", tag="ss")
                            nc.tensor.matmul(
                                psa[:, off:SBLK],
                                kT2[0:64, t * 128 : (t + 1) * 128],
                                qp[hp][0:64, b * SBLK + off : (b + 1) * SBLK],
                                start=True,
                                stop=True,
                                tile_position=(0, 0),
                            )
                            nc.tensor.matmul(
                                psb[:, off:SBLK],
                                kT2[64:128, t * 128 : (t + 1) * 128],
                                qp[hp][64:128, b * SBLK + off : (b + 1) * SBLK],
                                start=True,
                                stop=True,
                                tile_position=(64, 0),
                            )
                            for lane, pss in ((0, psa), (1, psb)):
                                ptt = PTP.tile([128, SBLK], BF16, name="ptt", tag="pt")
                                nc.scalar.activation(
                                    ptt[:, off:SBLK], pss[:, off:SBLK], AF.Exp,
                                    scale=SCALE,
                                )
                                if t >= 4 * b:
                                    nc.vector.tensor_tensor(
                                        ptt[:, off : off + 128],
                                        ptt[:, off : off + 128],
                                        tri_t[:],
                                        op=OP.mult,
                                    )
                                ptts.append(ptt)
                        for h in range(QH):
                            nc.tensor.matmul(
                                pso[h][:, off:SBLK],
                                vext[:, t * 65 : (t + 1) * 65],
                                ptts[h][:, off:SBLK],
                                start=(t == 0),
                                stop=(t == nt - 1),
                            )
                    # sink renorm via exp(-ln r) + K=1 bcast matmul
                    rowb = RP.tile([128, SBLK], F32, name="rowb", tag="rowb")
                    nc.gpsimd.memset(rowb[:], 1.0)
                    for h in range(QH):
                        nc.vector.tensor_scalar_add(
                            rowb[32 * h : 32 * h + 1, :],
                            pso[h][64:65, :],
                            esink_t[64:65, h : h + 1],
                        )
                    rinvb = RP.tile([128, SBLK], F32, name="rinvb", tag="rinvb")
                    nc.scalar.activation(rinvb[:], rowb[:], AF.Ln)
                    nc.scalar.activation(rowb[:], rinvb[:], AF.Exp, scale=-1.0)
                    for h in range(QH):
                        qb = (h % 2) * 64
                        ps_rb = AUX.tile([64, SBLK], F32, name="ps_rb", tag="aux")
                        nc.tensor.matmul(
                            ps_rb[:], ones_ft[32 * h : 32 * h + 1, 0:64],
                            rowb[32 * h : 32 * h + 1, :],
                            start=True, stop=True,
                            tile_position=(32 * h, 0),
                        )
                        rb = RBP.tile([64, SBLK], F32, name="rb", tag="rb")
                        nc.vector.tensor_copy(rb[:], ps_rb[:])
                        nc.vector.tensor_tensor(
                            outstk[h // 2][qb : qb + 64, b * SBLK : (b + 1) * SBLK],
                            pso[h][0:64, :],
                            rb[:],
                            op=OP.mult,
                        )
                    # output projection for this block's 4 sq tiles
                    for st in range(4 * b, 4 * b + 4):
                        for db in range(NSB):
                            ds = slice(db * SBLK, (db + 1) * SBLK)
                            psf = AUX.tile([128, SBLK], F32, name="psf", tag="aux")
                            nc.tensor.matmul(
                                psf[:],
                                outstk[0][:, st * 128 : (st + 1) * 128],
                                wo_t[0][:, ds],
                                start=True,
                                stop=False,
                            )
                            nc.tensor.matmul(
                                psf[:],
                                outstk[1][:, st * 128 : (st + 1) * 128],
                                wo_t[1][:, ds],
                                start=False,
                                stop=True,
                            )
                            ot = OE.tile([128, SBLK], F32, name="ot", tag="oe")
                            nc.vector.tensor_tensor(
                                ot[:], psf[:], biasb[:, ds], op=OP.add
                            )
                            nc.sync.dma_start(
                                out[st * 128 : (st + 1) * 128, ds], ot[:]
                            )
                    if b == 0:
                        for t in range(12, 16):
                            pv2 = AUX.tile([128, HD], BF16, name="pv2", tag="aux")
                            nc.tensor.transpose(
                                pv2[:], vT[:, t * 128 : (t + 1) * 128], idp_t[:]
                            )
                            nc.vector.tensor_copy(
                                vext[:, t * 65 : t * 65 + 64], pv2[:]
                            )
                            nc.vector.tensor_copy(
                                vext[:, t * 65 + 64 : t * 65 + 65], onesb_t[:]
                            )

    _fix_range_clears(nc)
    if split_waits:
        _split_excess_waits(nc)
    return nc


_nc_cache = [None]


def kernel(**inputs):
    in_maps = prep_inputs(inputs)
    if _nc_cache[0] is None:
        _nc_cache[0] = build_nc()
    nc = _nc_cache[0]
    res = run_bass_kernel_spmd(nc, in_maps, list(range(NCORES)))
    acc = res.results[0]["out"].astype(np.float32)
    for i in range(1, NCORES):
        acc = acc + res.results[i]["out"]
    return acc.reshape(B, S, DIM)
